# revision 1
# baseline (speedup 1.0000x reference)
"""Based-style linear attention (Taylor feature map) on 8 Trainium2 cores.

Math: reference computes, per head h (FDIM=16, HEAD_DIM=64):
    q,k = HS@Wq, HS@Wk    (per-head 16 dims), v = HS@Wv (per-head 64 dims)
    phi(x) = [1, x/2, outer(x,x)/(sqrt(2)*4)]      (273 dims)
    y_t = sum_{s<=t} (phi(q_t).phi(k_s)) v_s / sum_{s<=t} phi(q_t).phi(k_s)
    out = concat_h(y) @ Wo
Key identity: phi(q).phi(k) = Square(q.k/sqrt(32) + 1/sqrt(2)) + 1/2, so
scores come from 16-dim dot products + one Square; the 273-dim feature map
is never materialized.

Sharding: head-parallel, no collectives. 16 virtual heads (12 real + 4 zero
dummies), 2 per core. Host sums the 8 partial outputs.

Layout/perf notes (v3):
 - scale+bias folded into the score matmul: Wq/Wk pre-scaled by 32^-1/4 on
   host, and a constant row (1 on the k side, 1/sqrt(2) on the q side) at
   partition 0 (h0) / 32 (h1) of the kq tile, so the score contraction is
   K=17 and the activation is a plain Square (runs on ACT for h0, DVE h1).
 - q and k projected in ONE pass over hs: stationary [128, 113] covers
   k(h0,h1)+q(h0,h1) with zero padding placed so all PSUM->SBUF copies and
   memsets start at 32-aligned partitions (engine APs need 32-alignment).
 - intra-chunk +1/2 term folded into the diag mask: (sq+0.5)*tri via one
   gpsimd scalar_tensor_tensor -- no htri matmuls.
 - den replicated across 64 PSUM rows: V-stationary is [v(64) | ones(64)],
   so num rows 0-63 / den rows 64-127 come from one matmul and
   y = num * recip(den) is a plain elementwise mul -- no broadcast matmul.
 - packed output projection: yT for both heads stacked [128, L]; one
   128-contraction matmul per row chunk (6144 streamed cols, was 12288).
 - pipelined finalization: after kv chunk j in {1,3,5,7}, query region
   [256r, 256r+256) is complete; recip/yT/o-proj/DMA-out for it overlap the
   remaining attention chunks instead of serializing at the end.
 - batched input DMA: 4 triggers on sync (weights, hs 2+4 chunks, consts)
   + wo on scalar, ordered so the first projection matmul starts asap.
"""

import math

import numpy as np
import ml_dtypes

import concourse.bass as bass
import concourse.mybir as mybir
import concourse.tile as tile
from concourse import bacc
from concourse.bass_utils import run_bass_kernel_spmd

L = 1024
D = 768
H = 12
FD = 16
HD = 64
NCORE = 8
NCH = 8  # L chunks of 128
KB = 6  # contraction blocks of 128 over D
F32 = mybir.dt.float32
BF16 = mybir.dt.bfloat16

DT = BF16  # on-chip compute dtype (PE streams 1 col/cycle bf16 vs 1/4 fp32)

A_BIAS = 1.0 / math.sqrt(2.0)
S4 = 32.0 ** -0.25  # folded into Wq and Wk on host

# wts columns per kb: kq block 113 wide (see _host_weights), then wv 128 wide
KQW = 113
WV_OFF = KQW * KB  # 678

_compiled_nc = None
_last_in_maps = None


def _splits(lo, hi, step):
    out = []
    a = lo
    while a < hi:
        b = min(hi, (a // step + 1) * step)
        out.append((a, b))
        a = b
    return out


def _build_nc():
    nc = bacc.Bacc("TRN2", target_bir_lowering=False, debug=False, num_devices=NCORE)

    wts = nc.dram_tensor("wts", [128, WV_OFF + 128 * KB], DT, kind="ExternalInput")
    hsd = nc.dram_tensor("hsd", [D, L], DT, kind="ExternalInput")
    # consts packed: tri 0:128 | ones8 128:192 | sel rows 0-7 at 192:1216
    c2 = nc.dram_tensor("c2", [128, 1216], DT, kind="ExternalInput")
    wo = nc.dram_tensor("wo", [128, D], DT, kind="ExternalInput")
    out = nc.dram_tensor("out", [L, D], DT, kind="ExternalOutput")

    hs_re = hsd.ap().rearrange("(po pi) f -> pi po f", pi=128)

    with tile.TileContext(nc) as tc:
        with (
            tc.tile_pool(name="cst", bufs=1) as cst,
            tc.tile_pool(name="sqp", bufs=4) as sqp,
            tc.tile_pool(name="wrk", bufs=2) as wrk,
        ):
            # ---- input DMA: one trigger per ~128-256KB so transfers land
            # on parallel DMA queues (a single trigger's descriptors run on
            # ONE queue at ~75 GB/s), spread across the sync+scalar DGEs,
            # first-needed first ----
            wts_sb = cst.tile([128, WV_OFF + 128 * KB], DT, tag="wts")
            hs01 = cst.tile([128, 2, 1024], DT, tag="hs01")
            hs25 = cst.tile([128, 4, 1024], DT, tag="hs25")
            c2_sb = cst.tile([128, 1216], DT, tag="c2")
            wo_sb = cst.tile([128, D], DT, tag="wo")
            nc.sync.dma_start(out=wts_sb[:, 0 : 2 * KQW], in_=wts.ap()[:, 0 : 2 * KQW])
            nc.sync.dma_start(out=hs01[:, 0, 0:512], in_=hs_re[:, 0, 0:512])
            nc.sync.dma_start(out=hs01[:, 0, 512:1024], in_=hs_re[:, 0, 512:1024])
            nc.sync.dma_start(out=hs01[:, 1, :], in_=hs_re[:, 1, :])
            nc.sync.dma_start(out=hs25[:, 0, :], in_=hs_re[:, 2, :])
            nc.sync.dma_start(out=c2_sb, in_=c2.ap())
            nc.scalar.dma_start(
                out=wts_sb[:, 2 * KQW : WV_OFF], in_=wts.ap()[:, 2 * KQW : WV_OFF]
            )
            nc.scalar.dma_start(out=wts_sb[:, WV_OFF:], in_=wts.ap()[:, WV_OFF:])
            nc.scalar.dma_start(out=hs25[:, 1, :], in_=hs_re[:, 3, :])
            nc.scalar.dma_start(out=hs25[:, 2, :], in_=hs_re[:, 4, :])
            nc.scalar.dma_start(out=hs25[:, 3, :], in_=hs_re[:, 5, :])
            nc.scalar.dma_start(out=wo_sb, in_=wo.ap())

            def hs_kb(kb):
                return hs01[:, kb, :] if kb < 2 else hs25[:, kb - 2, :]

            tri_sb = c2_sb[:, 0:128]
            ones8_sb = c2_sb[:, 128:192]
            sel_sb = c2_sb[0:8, 192:1216]

            # kq: row 0 const / rows 1-16 h0 dims, row 32 const / rows 33-48
            # h1 dims; cols 0-1023 k^T, 1024-2047 q^T
            kq_sb = cst.tile([49, 2048], DT, tag="kq")
            # vx: [part, chunk, head, v(64)|ones(64)]
            vx_sb = cst.tile([128, NCH, 2, 128], DT, tag="vx")
            colsum_sb = cst.tile([8, 2, 128], DT, tag="colsum")
            yT_sb = cst.tile([128, L], DT, tag="yT")

            # bias vectors that materialize the const rows (1 on the k side,
            # A_BIAS on the q side) during the PSUM->SBUF copies: PSUM rows
            # 0/32 are zero (zero weight cols), Identity adds the bias there
            bias_k = cst.tile([49, 1], F32, tag="bias_k")
            bias_q = cst.tile([49, 1], F32, tag="bias_q")
            nc.vector.memset(bias_k, 0.0)
            nc.vector.memset(bias_k[0:1, :], 1.0)
            nc.vector.memset(bias_k[32:33, :], 1.0)
            nc.vector.memset(bias_q, 0.0)
            nc.vector.memset(bias_q[0:1, :], A_BIAS)
            nc.vector.memset(bias_q[32:33, :], A_BIAS)

            # ================= projections =================
            with tc.tile_pool(name="ps1", bufs=1, space="PSUM") as ps1:
                # q/k -> kq_sb in one pass: PSUM rows 0-48 k-block (const-row
                # gaps zero), rows 64-112 q-block.  kb outer so the matmuls
                # consume hs chunks in DMA-arrival order.
                pqk = [
                    ps1.tile([113, 512], F32, tag=f"pqk{half}", name=f"pqk{half}")
                    for half in range(2)
                ]
                for kb in range(KB):
                    for half in range(2):
                        nc.tensor.matmul(
                            pqk[half],
                            wts_sb[:, kb * KQW : (kb + 1) * KQW],
                            hs_kb(kb)[:, half * 512 : (half + 1) * 512],
                            start=(kb == 0),
                            stop=(kb == KB - 1),
                        )
                for half in range(2):
                    p = pqk[half]
                    cols = slice(half * 512, (half + 1) * 512)
                    nc.scalar.activation(
                        out=kq_sb[0:49, cols],
                        in_=p[0:49],
                        func=mybir.ActivationFunctionType.Identity,
                        bias=bias_k[:, 0:1],
                    )
                    nc.scalar.activation(
                        out=kq_sb[0:49, 1024 + half * 512 : 1024 + (half + 1) * 512],
                        in_=p[64:113],
                        func=mybir.ActivationFunctionType.Identity,
                        bias=bias_q[:, 0:1],
                    )

                # v -> vx_sb
                with tc.tile_pool(name="psv", bufs=2, space="PSUM") as psv:
                    for ch in range(NCH):
                        pv = psv.tile([128, 2, 64], F32, tag="pv", name=f"pv{ch}")
                        for kb in range(KB):
                            nc.tensor.matmul(
                                pv,
                                hs_kb(kb)[:, ch * 128 : (ch + 1) * 128],
                                wts_sb[:, WV_OFF + kb * 128 : WV_OFF + (kb + 1) * 128],
                                start=(kb == 0),
                                stop=(kb == KB - 1),
                            )
                        nc.vector.tensor_copy(vx_sb[:, ch, :, 0:64], pv)
                    nc.gpsimd.memset(vx_sb[:, :, :, 64:128], 1.0)

                    # per-chunk column sums of vx (inter-chunk +1/2 term)
                    pcs = ps1.tile([8, 2, 128], F32, tag="pcs", name="pcs")
                    for ch in range(NCH):
                        nc.tensor.matmul(
                            pcs,
                            ones8_sb[:, ch * 8 : (ch + 1) * 8],
                            vx_sb[:, ch, :, :],
                            start=(ch == 0),
                            stop=(ch == NCH - 1),
                        )
                    nc.vector.tensor_copy(colsum_sb, pcs)

            # ================= attention =================
            with (
                tc.tile_pool(name="psn", bufs=1, space="PSUM") as psn,
                tc.tile_pool(name="psa", bufs=2, space="PSUM") as psa,
                tc.tile_pool(name="pso", bufs=1, space="PSUM") as pso,
            ):
                nums = [
                    psn.tile([128, L], F32, tag=f"pN{h}", name=f"num{h}")
                    for h in range(2)
                ]
                pending = []  # o-proj chunks finalized, not yet emitted

                def emit_oproj():
                    while pending:
                        i = pending.pop(0)
                        po = pso.tile([128, 768], F32, tag="po", name=f"po{i}")
                        for a, b in ((0, 512), (512, 768)):
                            nc.tensor.matmul(
                                po[:, a:b],
                                yT_sb[:, i * 128 : (i + 1) * 128],
                                wo_sb[:, a:b],
                                start=True,
                                stop=True,
                            )
                        # split the PSUM->SBUF copy across ACT+DVE so the po
                        # bank frees quickly (pso has bufs=1); ACT is the
                        # hotter engine so it gets the smaller piece
                        osb = wrk.tile([128, D], DT, tag="osb")
                        nc.scalar.activation(
                            out=osb[:, 0:256],
                            in_=po[:, 0:256],
                            func=mybir.ActivationFunctionType.Copy,
                        )
                        nc.vector.tensor_copy(osb[:, 256:768], po[:, 256:768])
                        if i >= 6:
                            # last chunks sit on the critical tail: use two
                            # DMA queues each
                            nc.sync.dma_start(
                                out=out.ap()[i * 128 : (i + 1) * 128, 0:384],
                                in_=osb[:, 0:384],
                            )
                            nc.sync.dma_start(
                                out=out.ap()[i * 128 : (i + 1) * 128, 384:768],
                                in_=osb[:, 384:768],
                            )
                        else:
                            nc.sync.dma_start(
                                out=out.ap()[i * 128 : (i + 1) * 128, :], in_=osb
                            )

                for j in range(NCH):
                    tlo = j * 128
                    width = L - tlo
                    sqs = []
                    # scores for BOTH heads first: h1's score matmul fills
                    # the PE queue while h0's square+mask run on ACT/gpsimd
                    for h in range(2):
                        rb = 32 * h
                        sq = sqp.tile([128, 1024], DT, tag="sq", name=f"sq{j}_{h}")[
                            :, :width
                        ]
                        sqs.append(sq)
                        for a, b in _splits(tlo, L, 512):
                            pa = psa.tile([128, 512], F32, tag="pa", name=f"pa{j}{h}{a}")[
                                :, : b - a
                            ]
                            nc.tensor.matmul(
                                pa,
                                kq_sb[rb : rb + 17, tlo : tlo + 128],
                                kq_sb[rb : rb + 17, 1024 + a : 1024 + b],
                                start=True,
                                stop=True,
                            )
                            # ACT is the only engine that can square straight
                            # from PSUM (DVE would need two PSUM reads)
                            nc.scalar.square(out=sq[:, a - tlo : b - tlo], in_=pa)
                        # diag block: (sq + 1/2) * tri -- folds the
                        # intra-chunk +1/2 term into the causal mask (DVE;
                        # gpsimd TensorTensor is ~3x slower and this is on
                        # the square->mask->AV critical path)
                        nc.vector.scalar_tensor_tensor(
                            out=sq[:, 0:128],
                            in0=sq[:, 0:128],
                            scalar=0.5,
                            in1=tri_sb,
                            op0=mybir.AluOpType.add,
                            op1=mybir.AluOpType.mult,
                        )
                    for h in range(2):
                        # num/den += V_j-stationary @ sq
                        for a, b in _splits(tlo, L, 512):
                            nc.tensor.matmul(
                                nums[h][:, a:b],
                                vx_sb[:, j, h, :],
                                sqs[h][:, a - tlo : b - tlo],
                                start=(j == 0),
                                stop=False,
                            )
                    emit_oproj()
                    if j % 2 == 1:
                        # query region [256r, 256r+256) is complete
                        r = j // 2
                        lo, hi = 256 * r, 256 * r + 256
                        for h in range(2):
                            nc.tensor.matmul(
                                nums[h][:, lo:hi],
                                colsum_sb[0:8, h, :],
                                sel_sb[:, lo:hi],
                                start=False,
                                stop=True,
                            )
                            rc = wrk.tile([128, 256], F32, tag="rc")
                            nc.vector.reciprocal_approx_fast(
                                out=rc, in_=nums[h][:, lo:hi]
                            )
                            nc.vector.tensor_mul(
                                yT_sb[64 * h : 64 * h + 64, lo:hi],
                                nums[h][0:64, lo:hi],
                                rc[64:128, :],
                            )
                        pending.extend([2 * r, 2 * r + 1])
                emit_oproj()

    nc.finalize()
    return nc


def _host_consts():
    s = np.arange(128)[:, None]
    t = np.arange(128)[None, :]
    tri = (s <= t).astype(np.float32)
    sel = np.zeros((8, 1024), dtype=np.float32)
    for i in range(8):
        sel[:i, i * 128 : (i + 1) * 128] = 0.5
    ones8 = np.zeros((128, 64), dtype=np.float32)
    for ch in range(8):
        ones8[:, ch * 8 + ch] = 1.0
    return tri, sel, ones8


def kernel(hidden_states, Wq, Wk, Wv, Wo):
    global _compiled_nc, _last_in_maps
    hs = np.asarray(hidden_states, dtype=np.float32)[0]  # [L, D]
    Wq = np.asarray(Wq, dtype=np.float32)
    Wk = np.asarray(Wk, dtype=np.float32)
    Wv = np.asarray(Wv, dtype=np.float32)
    Wo = np.asarray(Wo, dtype=np.float32)

    if _compiled_nc is None:
        _compiled_nc = _build_nc()
    nc = _compiled_nc

    npdt = ml_dtypes.bfloat16
    hsT = np.ascontiguousarray(hs.T).astype(npdt)  # [D, L]
    tri, sel, ones8 = _host_consts()
    c2 = np.zeros((128, 1216), dtype=np.float32)
    c2[:, 0:128] = tri
    c2[:, 128:192] = ones8
    c2[0:8, 192:1216] = sel
    c2 = c2.astype(npdt)

    in_maps = []
    for c in range(NCORE):
        heads = [2 * c, 2 * c + 1]
        # kq weight block [768, 113]: cols 1-16 k_h0, 33-48 k_h1,
        # 65-80 q_h0, 97-112 q_h1 (gaps zero => aligned const rows/copies)
        kqw = np.zeros((D, KQW), dtype=np.float32)
        wv_c = np.zeros((D, 128), dtype=np.float32)
        wo_c = np.zeros((128, D), dtype=np.float32)
        for hi, h in enumerate(heads):
            if h >= H:
                continue
            kqw[:, 1 + 32 * hi : 1 + 32 * hi + FD] = Wk[:, h * FD : (h + 1) * FD] * S4
            kqw[:, 65 + 32 * hi : 65 + 32 * hi + FD] = Wq[:, h * FD : (h + 1) * FD] * S4
            wv_c[:, 64 * hi : 64 * hi + HD] = Wv[:, h * HD : (h + 1) * HD]
            wo_c[64 * hi : 64 * hi + HD, :] = Wo[h * HD : (h + 1) * HD, :]
        wts_c = np.zeros((128, WV_OFF + 128 * KB), dtype=np.float32)
        for kb in range(KB):
            blk = slice(kb * 128, (kb + 1) * 128)
            wts_c[:, kb * KQW : (kb + 1) * KQW] = kqw[blk]
            wts_c[:, WV_OFF + kb * 128 : WV_OFF + (kb + 1) * 128] = wv_c[blk]
        in_maps.append(
            {
                "wts": wts_c.astype(npdt),
                "hsd": hsT,
                "c2": c2,
                "wo": wo_c.astype(npdt),
            }
        )

    _last_in_maps = in_maps
    res = run_bass_kernel_spmd(nc, in_maps, list(range(NCORE)))
    acc = np.zeros((L, D), dtype=np.float32)
    for c in range(NCORE):
        acc += np.asarray(res.results[c]["out"], dtype=np.float32)
    return acc.reshape(1, L, D)

